# revision 24
# baseline (speedup 1.0000x reference)
"""COPNLL loss kernel for Trainium2 (8 NeuronCores).

Math: the reference builds V = (sig2e*I + sig2bs0*Z0 Z0^T + sig2bs1*Z1 Z1^T)/sig2
with Z0 (4096x1000), Z1 (4096x500) one-hot, then needs logdet(V) and m^T V^-1 m.
Both reduce via Woodbury to the 1500x1500 capacitance matrix
    K = [[sig2e/s0*I + diag(c0), C], [C^T, sig2e/s1*I + diag(c1)]]
whose (0,0) block is diagonal, leaving one dense 500x500 Schur complement
    S = (sig2e/s1*I + diag(c1)) - C^T diag(1/A) C,   A = sig2e/s0 + c0
with C = Z0^T Z1 (co-occurrence counts), c0/c1 level counts, a = Z0^T m, b = Z1^T m:
    logdet(sig2*V) = (N-q)log sig2e + q0 log s0 + q1 log s1 + sum(log A) + logdet S
    m^T V^-1 m     = (sig2/sig2e) * (m^T m - a^T A^-1 a - t^T S^-1 t),
                     t = b - C^T (a/A)
Device plan (SPMD on 8 cores):
  phase A (row-sharded, 512 rows/core): one-hot chunks via iota+is_equal;
    an early pass computes G1 = [1|m]^T [Z1|1|m] and G0a = [1|m]^T Z0, whose
    f32 all-reduce overlaps the main C = Z0^T Z1 matmuls; C is all-reduced
    in int8 (entries are small co-occurrence counts - exact).
  phase C (redundant on all cores): assemble S (padded to 512), block LDL with
    128-blocks; block inverses via Hotelling/Newton-Schulz Y-chains (spectrum
    of S ~ [1.7,17]) applied directly to the trailing panel; block logdets via
    Chebyshev trace of log with the weighted sum accumulated on the PE.
"""

import math
import sys
import types

import numpy as np

import concourse.bass as bass
import concourse.bacc as bacc
import concourse.mybir as mybir
from concourse.bass import ds, ts
from concourse.bass_utils import run_bass_kernel_spmd
from concourse.masks import make_identity
from concourse.tile import TileContext


def _ensure_axon_hooks():
    """bass_utils imports antenv.axon_hooks when tracing; this image's antenv
    lacks it. Provide a shim (with the real ctypes NTFF hook when available)
    so trace=True/BASS_TRACE never crashes the kernel."""
    try:
        import antenv.axon_hooks  # noqa: F401
        return
    except ImportError:
        pass
    try:
        import trn_agent_boot.trn_boot as tb
        hook = tb._ntff_profile_via_ctypes("/opt/axon/libaxon_pjrt.so")
    except Exception:
        hook = None
    mod = types.ModuleType("antenv.axon_hooks")
    mod._hook = hook
    mod.get_axon_ntff_profile_hook = lambda: mod._hook

    def _set(h):
        mod._hook = h

    mod.set_axon_ntff_profile_hook = _set
    sys.modules["antenv.axon_hooks"] = mod
    try:
        import antenv
        antenv.axon_hooks = mod
    except ImportError:
        pass
    try:
        import concourse.bass_utils as bu
        _orig_upload = bu.upload_artifacts

        def _safe_upload(tmpdir):
            try:
                return _orig_upload(tmpdir)
            except Exception:
                return f"local:{tmpdir}"

        bu.upload_artifacts = _safe_upload
    except Exception:
        pass


_ensure_axon_hooks()

N = 4096
NCORES = 8
Q0, Q0P = 1000, 1024
T0 = Q0P // 128            # 8 level-0 tiles of 128
Q1 = 500
FR = Q1 + 2                # rhs width: [Z1 | 1 | m]
SP = 512                   # padded S size
NBLK = SP // 128           # 4
W3 = Q1 - 3 * 128          # 116: valid width of the last S block
LO, HI = 1.4, 18.0         # eigenvalue bounds for NS init + Chebyshev interval
NS_ITERS = 5
CHEB_DEG = 12
NCOEF = CHEB_DEG + 1
CLIP = 4.2648907939226017  # sqrt(2)*erfinv(1-2e-5)

F32 = mybir.dt.float32
BF16 = mybir.dt.bfloat16
I8 = mybir.dt.int8
I32 = mybir.dt.int32
AX = mybir.AxisListType
OP = mybir.AluOpType
ACT = mybir.ActivationFunctionType

RED_C = 128 * T0 * Q1           # int8 elements: the C matrix
REDF_G0A = 2 * Q0P              # f32: [counts0 ; a] rows
REDF_G1 = REDF_G0A + 2 * FR     # f32: + G1
REDF_N = REDF_G1 + 2            # f32: + scalars


def cheb_coeffs(lo=LO, hi=HI, deg=CHEB_DEG):
    K = 4000
    th = (np.arange(K) + 0.5) * np.pi / K
    xk = np.cos(th)
    fk = np.log((hi - lo) / 2.0 * xk + (hi + lo) / 2.0)
    cs = np.array([2.0 / K * np.sum(fk * np.cos(j * th)) for j in range(deg + 1)])
    cs[0] *= 0.5
    return cs.astype(np.float32)


def _diag_fill(nc, tile_ap, value):
    nc.gpsimd.memset(tile_ap, 0.0)
    nc.gpsimd.affine_select(out=tile_ap, in_=tile_ap, compare_op=OP.not_equal,
                            fill=value, base=0, pattern=[[-1, 128]],
                            channel_multiplier=1)


def build_module(n_cores=NCORES):
    rows = N // n_cores
    nch = rows // 128          # 128-row chunks per core

    nc = bacc.Bacc(num_devices=n_cores)
    pk_d = nc.declare_dram_parameter("packed", [128, 4 * (rows // 128)], F32,
                                     isOutput=False)
    cst_d = nc.declare_dram_parameter("consts", [16], F32, isOutput=False)
    chb_d = nc.declare_dram_parameter("chebc", [2 * NCOEF], F32, isOutput=False)
    out_d = nc.declare_dram_parameter("out", [1, 1], F32, isOutput=True)

    redb_in = nc.dram_tensor("redb_in", [RED_C], I8)
    redb_out = nc.dram_tensor("redb_out", [RED_C], I8, addr_space="Shared")
    redf_in = nc.dram_tensor("redf_in", [REDF_N], F32)
    redf_out = nc.dram_tensor("redf_out", [REDF_N], F32, addr_space="Shared")
    warm_in = nc.dram_tensor("warm_in", [16], F32)
    warm_out = nc.dram_tensor("warm_out", [16], F32, addr_space="Shared")

    with TileContext(nc) as tc, \
         tc.tile_pool(name="consts", bufs=1) as consts, \
         tc.tile_pool(name="work", bufs=1) as work:

        # ---- constants ----
        ident = consts.tile([128, 128], F32, tag="ident")
        make_identity(nc, ident)
        i2 = consts.tile([128, 128], F32, tag="i2")              # 2*I
        _diag_fill(nc, i2, 2.0)
        alphaI = consts.tile([128, 128], F32, tag="alphaI")      # NS init
        _diag_fill(nc, alphaI, 2.0 / (LO + HI))
        shiftI = consts.tile([128, 128], F32, tag="shiftI")      # Chebyshev shift
        _diag_fill(nc, shiftI, (HI + LO) / (HI - LO))
        ones512 = consts.tile([128, SP], F32, tag="ones512")
        nc.vector.memset(ones512, 1.0)
        identB16 = consts.tile([128, 128], BF16, tag="identB16")
        nc.gpsimd.tensor_copy(identB16, ident)

        cst_row = consts.tile([1, 16], F32, tag="cst_row")
        nc.sync.dma_start(cst_row, cst_d[:].rearrange("(p x) -> p x", p=1))
        cst_row2 = consts.tile([1, 16], F32, tag="cst_row2")
        nc.vector.tensor_copy(cst_row2, cst_row)
        chb = consts.tile([1, 2 * NCOEF], F32, tag="chb")
        nc.sync.dma_start(chb, chb_d[:].rearrange("(p x) -> p x", p=1))
        chb2 = consts.tile([1, 2 * NCOEF], F32, tag="chb2")
        nc.vector.tensor_copy(chb2, chb)
        cst = consts.tile([128, 16], F32, tag="cst")
        chbB = consts.tile([128, 2 * NCOEF], F32, tag="chbB")
        with tc.tile_pool(name="setup_ps", bufs=2,
                          space=bass.MemorySpace.PSUM) as gps0:
            ps_b = gps0.tile([128, 16], F32, tag="gps0")
            nc.tensor.matmul(ps_b, ones512[0:1, 0:128], cst_row2,
                             start=True, stop=True)
            nc.vector.tensor_copy(cst, ps_b)
            ps_c = gps0.tile([128, 2 * NCOEF], F32, tag="gps0")
            nc.tensor.matmul(ps_c, ones512[0:1, 0:128], chb2,
                             start=True, stop=True)
            nc.vector.tensor_copy(chbB, ps_c)

        # iotas for the one-hot compares
        iota0i = work.tile([128, Q0P], I32, tag="iota0i")
        nc.gpsimd.iota(iota0i, pattern=[[1, Q0P]], base=0, channel_multiplier=0)
        iota0 = work.tile([128, Q0P], F32, tag="iota0")
        nc.gpsimd.tensor_copy(iota0, iota0i)
        iota1i = work.tile([128, Q1], I32, tag="iota1i")
        nc.gpsimd.iota(iota1i, pattern=[[1, Q1]], base=0, channel_multiplier=0)
        iota1 = work.tile([128, Q1], F32, tag="iota1")
        nc.gpsimd.tensor_copy(iota1, iota1i)

        # ---- inputs -> m, sum r^2, sum m^2 ----
        packed = work.tile([128, 4 * nch], F32, tag="packed")
        nc.sync.dma_start(packed, pk_d[:])
        yt = packed[:, 0:nch]
        yp = packed[:, nch:2 * nch]
        idx0 = work.tile([128, nch], F32, tag="idx0")
        nc.vector.tensor_copy(idx0, packed[:, 2 * nch:3 * nch].bitcast(I32))
        idx1 = work.tile([128, nch], F32, tag="idx1")
        nc.vector.tensor_copy(idx1, packed[:, 3 * nch:4 * nch].bitcast(I32))
        resid = work.tile([128, nch], F32, tag="resid")
        nc.vector.tensor_sub(resid, yt, yp)
        mvec = work.tile([128, nch], F32, tag="mvec")
        nc.vector.tensor_scalar(out=mvec, in0=resid, scalar1=cst[:, 0:1],
                                scalar2=cst[:, 1:2], op0=OP.mult, op1=OP.min)
        nc.vector.tensor_scalar(out=mvec, in0=mvec, scalar1=cst[:, 8:9],
                                scalar2=None, op0=OP.max)
        scr_n = work.tile([128, nch], F32, tag="scr_n")
        scal2 = work.tile([128, 2], F32, tag="scal2")
        nc.vector.tensor_mul(scr_n, resid, resid)
        nc.vector.tensor_reduce(scal2[:, 0:1], scr_n, AX.X, OP.add)
        nc.vector.tensor_mul(scr_n, mvec, mvec)
        nc.vector.tensor_reduce(scal2[:, 1:2], scr_n, AX.X, OP.add)
        scal2r = work.tile([1, 2], F32, tag="scal2r")
        with tc.tile_pool(name="sc_ps", bufs=1,
                          space=bass.MemorySpace.PSUM) as gpsc:
            ps_s = gpsc.tile([128, 2], F32, tag="gpsc")
            nc.tensor.matmul(ps_s[0:1, 0:2], ones512[:, 0:1], scal2,
                             start=True, stop=True)
            nc.vector.tensor_copy(scal2r, ps_s[0:1, 0:2])

        # ---- phase A pass 1: G1 = [1|m]^T [Z1|1|m], G0a = [1|m]^T Z0 ----
        G1 = work.tile([2, FR], F32, tag="G1")
        G0a = work.tile([2, Q0P], F32, tag="G0a")
        share = nch <= 4
        oh0_keep, rhs_keep = [], []
        with (
            tc.tile_pool(name="ph1", bufs=(1 if share else 2)) as ph1,
            tc.tile_pool(name="ph1_ps", bufs=1, space=bass.MemorySpace.PSUM) as pg1,
        ):
            psg1 = pg1.tile([2, FR], F32, tag="psg1")
            psa0 = pg1.tile([2, SP], F32, tag="psa0")
            psa1 = pg1.tile([2, SP], F32, tag="psa1")
            for c in range(nch):
                cc = c % 4
                pool_c = work if share else ph1
                oh = pool_c.tile([128, Q0P], BF16, tag=f"oh0_{cc if not share else c}",
                                 name=f"oh0w_{c if share else cc}")
                nc.vector.tensor_scalar(out=oh, in0=iota0,
                                        scalar1=idx0[:, c:c + 1],
                                        scalar2=None, op0=OP.is_equal)
                rh = pool_c.tile([128, FR], BF16, tag=f"rhs_{cc if not share else c}",
                                 name=f"rhsw_{c if share else cc}")
                nc.vector.tensor_scalar(out=rh[:, 0:Q1], in0=iota1,
                                        scalar1=idx1[:, c:c + 1],
                                        scalar2=None, op0=OP.is_equal)
                nc.vector.memset(rh[:, Q1:Q1 + 1], 1.0)
                nc.vector.tensor_copy(rh[:, Q1 + 1:FR], mvec[:, c:c + 1])
                if share:
                    oh0_keep.append(oh)
                    rhs_keep.append(rh)
                first, lastc = c == 0, c == nch - 1
                nc.tensor.matmul(psg1, rh[:, Q1:FR], rh, start=first, stop=lastc)
                nc.tensor.matmul(psa0, rh[:, Q1:FR], oh[:, 0:SP],
                                 start=first, stop=lastc)
                nc.tensor.matmul(psa1, rh[:, Q1:FR], oh[:, SP:Q0P],
                                 start=first, stop=lastc)
            nc.vector.tensor_copy(G1, psg1)
            nc.vector.tensor_copy(G0a[:, 0:SP], psa0)
            nc.vector.tensor_copy(G0a[:, SP:Q0P], psa1)

        # early f32 all-reduce: G0a + G1 + scalars (overlaps the C matmuls)
        nc.sync.dma_start(
            redf_in[0:REDF_G0A].rearrange("(p f) -> p f", p=2), G0a)
        nc.sync.dma_start(
            redf_in[REDF_G0A:REDF_G1].rearrange("(p f) -> p f", p=2), G1)
        nc.sync.dma_start(
            redf_in[REDF_G1:REDF_G1 + 2].rearrange("(p f) -> p f", p=1), scal2r)
        if n_cores > 1:
            nc.gpsimd.collective_compute(
                "AllReduce", OP.add,
                replica_groups=[list(range(n_cores))],
                ins=[redf_in[:]], outs=[redf_out[:]],
            )
        else:
            nc.sync.dma_start(redf_out[:], redf_in[:])

        # phase C constants built early (fills the collective wait)
        iotaLi = work.tile([128, T0], I32, tag="iotaLi")
        nc.gpsimd.iota(iotaLi, pattern=[[128, T0]], base=0, channel_multiplier=1)
        iotaL = work.tile([128, T0], F32, tag="iotaL")
        nc.vector.tensor_copy(iotaL, iotaLi)
        padmask = work.tile([128, T0], mybir.dt.uint32, tag="padmask")
        nc.vector.tensor_scalar(out=padmask, in0=iotaL, scalar1=float(Q0) - 0.5,
                                scalar2=None, op0=OP.is_gt)
        cIh, cIl = [], []
        for j in range(NCOEF):
            th_ = work.tile([128, 128], BF16, tag=f"cIh{j}", name=f"cIh{j}")
            nc.gpsimd.tensor_scalar_mul(th_, ident, chbB[:, j:j + 1])
            cIh.append(th_)
            tl_ = work.tile([128, 128], BF16, tag=f"cIl{j}", name=f"cIl{j}")
            nc.gpsimd.tensor_scalar_mul(
                tl_, ident, chbB[:, NCOEF + j:NCOEF + j + 1])
            cIl.append(tl_)

        # ---- phase A pass 2: C = Z0^T Z1 ----
        G0 = work.tile([128, T0, Q1], BF16, tag="G0")
        with (
            tc.tile_pool(name="phA", bufs=2) as pha,
            tc.tile_pool(name="phA_ps", bufs=1, space=bass.MemorySpace.PSUM) as pps,
        ):
            psa = [pps.tile([128, Q1], F32, tag=f"psa{t}", name=f"psa{t}")
                   for t in range(T0)]
            for c in range(nch):
                cc = c % 4
                if share:
                    oh, rh = oh0_keep[c], rhs_keep[c]
                else:
                    oh = pha.tile([128, Q0P], BF16, tag=f"oh0b_{cc}",
                                  name=f"oh0b_{cc}")
                    nc.vector.tensor_scalar(out=oh, in0=iota0,
                                            scalar1=idx0[:, c:c + 1],
                                            scalar2=None, op0=OP.is_equal)
                    rh = pha.tile([128, FR], BF16, tag=f"rhsb_{cc}",
                                  name=f"rhsb_{cc}")
                    nc.vector.tensor_scalar(out=rh[:, 0:Q1], in0=iota1,
                                            scalar1=idx1[:, c:c + 1],
                                            scalar2=None, op0=OP.is_equal)
                for t in range(T0):
                    nc.tensor.matmul(psa[t], oh[:, ts(t, 128)], rh[:, 0:Q1],
                                     start=(c == 0), stop=(c == nch - 1))
            for t in range(T0):
                nc.vector.tensor_copy(G0[:, t, :], psa[t])

        # ---- int8 all-reduce of C (co-occurrence counts: exact) ----
        C8 = work.tile([128, T0, Q1], I8, tag="C8")
        nc.vector.tensor_copy(C8, G0)
        nc.sync.dma_start(
            redb_in[:].rearrange("(p t f) -> p t f", p=128, t=T0), C8)
        if n_cores > 1:
            nc.gpsimd.collective_compute(
                "AllReduce", OP.add,
                replica_groups=[list(range(n_cores))],
                ins=[redb_in[:]], outs=[redb_out[:]],
            )
        else:
            nc.sync.dma_start(redb_out[:], redb_in[:])
        # f32 results land first; their dependent work overlaps the int8 AR
        nc.sync.dma_start(
            G0a, redf_out[0:REDF_G0A].rearrange("(p f) -> p f", p=2))
        nc.sync.dma_start(
            G1, redf_out[REDF_G0A:REDF_G1].rearrange("(p f) -> p f", p=2))
        r2g = work.tile([1, 1], F32, tag="r2g")
        nc.sync.dma_start(
            r2g, redf_out[REDF_G1:REDF_G1 + 1].rearrange("(p f) -> p f", p=1))
        mtm = work.tile([1, 1], F32, tag="mtm")
        nc.sync.dma_start(
            mtm, redf_out[REDF_G1 + 1:REDF_G1 + 2].rearrange("(p f) -> p f", p=1))
        nc.sync.dma_start(
            C8, redb_out[:].rearrange("(p t f) -> p t f", p=128, t=T0))
        nc.vector.tensor_copy(G0, C8)

        # ---- phase C: S assembly ----
        Srow = [work.tile([128, SP], BF16, tag=f"Srow{i}", name=f"Srow{i}")
                for i in range(NBLK)]
        zvec = [work.tile([128, 1], F32, tag=f"z{i}", name=f"z{i}")
                for i in range(NBLK)]
        Avec = work.tile([128, T0], F32, tag="Avec")
        aAll = work.tile([128, T0], F32, tag="aAll")
        Winv = work.tile([128, T0], F32, tag="Winv")
        Cw = work.tile([128, T0, Q1], BF16, tag="Cw")

        with tc.tile_pool(name="sasm_ps", bufs=2,
                          space=bass.MemorySpace.PSUM) as sps:
            # counts0/a -> per-partition layout via PE transposes
            for t in range(T0):
                psT = sps.tile([128, 2], F32, tag="pst")
                nc.tensor.transpose(psT, G0a[0:2, ts(t, 128)], ident[0:2, 0:2])
                nc.vector.tensor_copy(Avec[:, t:t + 1], psT[:, 0:1])
                nc.vector.tensor_copy(aAll[:, t:t + 1], psT[:, 1:2])
            nc.vector.tensor_scalar(out=Avec, in0=Avec, scalar1=cst[:, 2:3],
                                    scalar2=None, op0=OP.add)
            nc.vector.copy_predicated(Avec, padmask, ones512[:, 0:T0])
            nc.vector.reciprocal(Winv, Avec)
            scr_t = work.tile([128, T0], F32, tag="scr_t")
            logA = work.tile([128, 1], F32, tag="logA")
            nc.scalar.activation(scr_t, Avec, ACT.Ln, accum_out=logA)
            for t in range(T0):
                nc.vector.tensor_scalar_mul(Cw[:, t, :], G0[:, t, :],
                                             Winv[:, t:t + 1])
            # quad_a = sum(a^2 / A)
            qa = work.tile([128, 1], F32, tag="qa")
            qscr = work.tile([128, T0], F32, tag="qscr")
            nc.vector.tensor_mul(qscr, aAll, aAll)
            nc.vector.tensor_mul(qscr, qscr, Winv)
            nc.vector.tensor_reduce(qa, qscr, AX.X, OP.add)
            aW = work.tile([128, T0], BF16, tag="aW")
            nc.vector.tensor_mul(aW, aAll, Winv)

            for i in range(NBLK):
                wi = 128 if i < NBLK - 1 else W3
                pss = sps.tile([128, Q1], F32, tag="pss", bufs=4)
                for t in range(T0):
                    nc.tensor.matmul(pss[:wi, :], Cw[:, t, ds(i * 128, wi)],
                                     G0[:, t, :], start=(t == 0),
                                     stop=(t == T0 - 1))
                # (C^T a/A)_i for the t vector
                psta = sps.tile([128, 1], F32, tag="psta", bufs=2)
                for t in range(T0):
                    nc.tensor.matmul(psta[:wi, :], G0[:, t, ds(i * 128, wi)],
                                     aW[:, t:t + 1],
                                     start=(t == 0), stop=(t == T0 - 1))
                nc.vector.memset(Srow[i], 0.0)
                nc.vector.tensor_scalar_mul(Srow[i][:wi, 0:Q1], pss[:wi, 0:Q1],
                                            -1.0)
                # c1/b block via PE transpose of G1[0:2, block]
                psT = sps.tile([128, 2], F32, tag="pst")
                nc.tensor.transpose(psT[:wi, :], G1[0:2, ds(i * 128, wi)],
                                    ident[0:2, 0:2])
                cbt = work.tile([128, 2], F32, tag=f"cb{i}", name=f"cb{i}")
                nc.vector.memset(cbt, 0.0)
                nc.vector.tensor_copy(cbt[:wi, :], psT[:wi, :])
                dS = work.tile([128, 1], F32, tag=f"dS{i}", name=f"dS{i}")
                nc.vector.tensor_scalar(out=dS, in0=cbt[:, 0:1],
                                        scalar1=cst[:, 3:4],
                                        scalar2=None, op0=OP.add)
                if i == NBLK - 1:
                    pm3 = work.tile([128, 1], mybir.dt.uint32, tag="pm3")
                    nc.vector.tensor_scalar(out=pm3, in0=iotaL[:, 0:1],
                                            scalar1=float(W3) - 0.5,
                                            scalar2=None, op0=OP.is_gt)
                    nc.vector.copy_predicated(dS, pm3, ones512[:, 0:1])
                dgblk = work.tile([128, 128], BF16, tag="dgblk")
                nc.vector.tensor_scalar_mul(dgblk, ident, dS)
                nc.vector.tensor_add(Srow[i][:, ts(i, 128)],
                                     Srow[i][:, ts(i, 128)], dgblk)

                nc.vector.memset(zvec[i], 0.0)
                nc.vector.tensor_sub(zvec[i][:wi, :], cbt[:wi, 1:2],
                                     psta[:wi, 0:1])

        # ---- block LDL: Hotelling chains + deferred Chebyshev traces ----
        Binv = [work.tile([128, 128], F32, tag=f"Binv{k}", name=f"Binv{k}")
                for k in range(NBLK)]
        Wk = [work.tile([128, SP - (k + 1) * 128], BF16, tag=f"Wk{k}",
                        name=f"Wk{k}") for k in range(NBLK - 1)]
        Wk32 = [work.tile([128, SP - (k + 1) * 128], F32, tag=f"Wk32_{k}",
                          name=f"Wk32_{k}") for k in range(NBLK - 1)]
        trc = work.tile([128, NBLK], F32, tag="trc")
        qtt = work.tile([128, NBLK], F32, tag="qtt")

        with (
            tc.tile_pool(name="ldl", bufs=4) as ldl,
            tc.tile_pool(name="ldl_ps", bufs=4, space=bass.MemorySpace.PSUM) as lps,
        ):
            alpha = 2.0 / (LO + HI)
            for k in range(NBLK):
                Bk = Srow[k][:, ts(k, 128)]
                # Hotelling: Y' = Y Z, Z = 2I - Y, Y0 = alpha*B -> Y -> I
                # X trails (-> B^-1); V trails on the panel (-> B^-1 Strail)
                trail = SP - (k + 1) * 128 if k < NBLK - 1 else 0
                Y = ldl.tile([128, 128], BF16, tag="nsY")
                nc.vector.tensor_scalar_mul(Y, Bk, alpha)
                Z = ldl.tile([128, 128], BF16, tag="nsZ")
                nc.vector.tensor_sub(Z, i2, Y)
                X = ldl.tile([128, 128], BF16, tag="nsX")
                nc.vector.tensor_copy(X, alphaI)
                psX = None
                for it in range(NS_ITERS):
                    last = it == NS_ITERS - 1
                    if not last:
                        psY = lps.tile([128, 128], F32, tag="lps")
                        nc.tensor.matmul(psY, Y, Z, start=True, stop=True)
                    psX = lps.tile([128, 128], F32, tag="lps")
                    nc.tensor.matmul(psX, X, Z, start=True, stop=True)
                    X = ldl.tile([128, 128], BF16, tag="nsX")
                    nc.vector.tensor_copy(X, psX)
                    if not last:
                        Z = ldl.tile([128, 128], BF16, tag="nsZ")
                        nc.vector.tensor_sub(Z, i2, psY)
                        Y = ldl.tile([128, 128], BF16, tag="nsY")
                        nc.vector.tensor_copy(Y, psY)
                nc.vector.tensor_copy(Binv[k], psX)
                if trail:
                    psW = lps.tile([128, 384], F32, tag="lps")
                    nc.tensor.matmul(psW[:, :trail], X,
                                     Srow[k][:, (k + 1) * 128:SP],
                                     start=True, stop=True)
                    nc.vector.tensor_copy(Wk[k], psW[:, :trail])
                    nc.vector.tensor_copy(Wk32[k], psW[:, :trail])
                    for i in range(k + 1, NBLK):
                        psu = lps.tile([128, 384], F32, tag="lps")
                        nc.tensor.matmul(psu[:, :trail], Srow[k][:, ts(i, 128)],
                                         Wk[k], start=True, stop=True)
                        nc.vector.tensor_sub(Srow[i][:, (k + 1) * 128:SP],
                                             Srow[i][:, (k + 1) * 128:SP],
                                             psu[:, :trail])

            # forward substitution: z_i -= (Wk[k] block i)^T z_k
            for k in range(NBLK - 1):
                for i in range(k + 1, NBLK):
                    psz = lps.tile([128, 1], F32, tag="lps")
                    off = (i - k - 1) * 128
                    nc.tensor.matmul(psz, Wk32[k][:, ds(off, 128)], zvec[k],
                                     start=True, stop=True)
                    nc.vector.tensor_sub(zvec[i], zvec[i], psz)
            # quad_t = sum_k z_k^T Binv_k z_k
            for k in range(NBLK):
                psq = lps.tile([128, 1], F32, tag="lps")
                nc.tensor.matmul(psq, Binv[k], zvec[k], start=True, stop=True)
                uk = ldl.tile([128, 1], F32, tag="uk")
                nc.vector.tensor_copy(uk, psq)
                nc.vector.tensor_mul(qtt[:, k:k + 1], zvec[k], uk)

            # Chebyshev trace chains, 4-wide interleaved; the weighted sum
            # R_k = sum_j c_j T_j accumulates on the PE via stationary c_j*I
            b2s, tprevs, tcurs, Rps = [], [], [], []
            for k in range(NBLK):
                Bk = Srow[k][:, ts(k, 128)]
                bh = ldl.tile([128, 128], BF16, tag=f"bh{k}", name=f"bh{k}")
                nc.vector.tensor_scalar_mul(bh, Bk, 2.0 / (HI - LO))
                nc.vector.tensor_sub(bh, bh, shiftI)
                b2 = ldl.tile([128, 128], BF16, tag=f"b2{k}", name=f"b2{k}")
                nc.vector.tensor_scalar_mul(b2, bh, 2.0)
                b2s.append(b2)
                tprev = ldl.tile([128, 128], BF16, tag=f"chT{k}",
                                 name=f"chTp{k}", bufs=3)
                nc.vector.tensor_copy(tprev, identB16)
                tprevs.append(tprev)
                tcurs.append(bh)
                R = lps.tile([128, 128], F32, tag="Rps", bufs=4,
                             name=f"Rps{k}")
                Rps.append(R)
                nc.tensor.matmul(R, cIh[0], identB16, start=True, stop=False)
                nc.tensor.matmul(R, cIl[0], identB16, start=False, stop=False)
                nc.tensor.matmul(R, cIh[1], bh, start=False, stop=False)
                nc.tensor.matmul(R, cIl[1], bh, start=False, stop=False)
            for j in range(2, CHEB_DEG + 1):
                for k in range(NBLK):
                    psc = lps.tile([128, 128], F32, tag="lps")
                    nc.tensor.matmul(psc, b2s[k], tcurs[k], start=True,
                                     stop=True)
                    tnext = ldl.tile([128, 128], BF16, tag=f"chT{k}",
                                     name=f"chT{k}_{j}", bufs=3)
                    nc.vector.tensor_sub(tnext, psc, tprevs[k])
                    nc.tensor.matmul(Rps[k], cIh[j], tnext, start=False,
                                     stop=False)
                    nc.tensor.matmul(Rps[k], cIl[j], tnext, start=False,
                                     stop=(j == CHEB_DEG))
                    tprevs[k], tcurs[k] = tcurs[k], tnext
            for k in range(NBLK):
                Rsb = ldl.tile([128, 128], F32, tag="Rsb")
                nc.vector.tensor_mul(Rsb, Rps[k], ident)   # keep diagonal only
                nc.vector.tensor_reduce(trc[:, k:k + 1], Rsb, AX.X, OP.add)

        # ---- final scalar assembly ----
        qtr = work.tile([128, 1], F32, tag="qtr")
        nc.vector.tensor_reduce(qtr, qtt, AX.X, OP.add)
        smalls_c = work.tile([128, 3 + NBLK], F32, tag="smalls_c")
        nc.vector.tensor_copy(smalls_c[:, 0:1], logA)
        nc.vector.tensor_copy(smalls_c[:, 1:2], qa)
        nc.vector.tensor_copy(smalls_c[:, 2:3], qtr)
        nc.vector.tensor_copy(smalls_c[:, 3:3 + NBLK], trc)
        smalls = work.tile([1, 3 + NBLK], F32, tag="smalls")
        ldS = work.tile([1, 1], F32, tag="ldS")
        with tc.tile_pool(name="fin_ps", bufs=1,
                          space=bass.MemorySpace.PSUM) as gps2:
            ps_sm = gps2.tile([128, 3 + NBLK], F32, tag="gps2")
            nc.tensor.matmul(ps_sm[0:1, :], ones512[:, 0:1], smalls_c,
                             start=True, stop=True)
            nc.vector.tensor_copy(smalls, ps_sm[0:1, :])
        nc.vector.tensor_reduce(ldS, smalls[:, 3:3 + NBLK], AX.X, OP.add)

        fin = work.tile([1, 8], F32, tag="fin")
        # quadK = quad_a + quad_t
        nc.vector.tensor_add(fin[:, 0:1], smalls[:, 1:2], smalls[:, 2:3])
        # mVinvm = (sig2/sig2e) * (mtm - quadK)
        nc.vector.tensor_sub(fin[:, 1:2], mtm, fin[:, 0:1])
        nc.vector.tensor_scalar_mul(fin[:, 1:2], fin[:, 1:2], cst[0:1, 6:7])
        # logdetV = const1 + sum log A + logdet S
        nc.vector.tensor_add(fin[:, 2:3], smalls[:, 0:1], ldS)
        nc.vector.tensor_scalar(out=fin[:, 2:3], in0=fin[:, 2:3],
                                scalar1=cst[0:1, 4:5], scalar2=None, op0=OP.add)
        # sum_log_pdf = const2 - sum_r2/(2 sig2)
        nc.vector.tensor_scalar(out=fin[:, 3:4], in0=r2g, scalar1=cst[0:1, 7:8],
                                scalar2=cst[0:1, 5:6], op0=OP.mult, op1=OP.add)
        # total = 0.5*(logdetV + mVinvm - mtm + sum_log_pdf)
        nc.vector.tensor_add(fin[:, 4:5], fin[:, 2:3], fin[:, 1:2])
        nc.vector.tensor_sub(fin[:, 4:5], fin[:, 4:5], mtm)
        nc.vector.tensor_add(fin[:, 4:5], fin[:, 4:5], fin[:, 3:4])
        nc.vector.tensor_scalar_mul(fin[:, 4:5], fin[:, 4:5], 0.5)

        nc.sync.dma_start(out_d[:], fin[:, 4:5])

    nc.finalize()
    return nc


def host_consts(sig2e, sig2bs):
    s0, s1 = float(sig2bs[0]), float(sig2bs[1])
    sig2e = float(sig2e)
    sig2 = sig2e + s0 + s1
    c = np.zeros(16, np.float32)
    c[0] = 1.0 / math.sqrt(sig2)
    c[1] = CLIP
    c[2] = sig2e / s0
    c[3] = sig2e / s1
    c[4] = ((N - Q0 - Q1) * math.log(sig2e) + Q0 * math.log(s0)
            + Q1 * math.log(s1) - N * math.log(sig2))
    c[5] = -0.5 * N * math.log(2.0 * math.pi * sig2)
    c[6] = sig2 / sig2e
    c[7] = -1.0 / (2.0 * sig2)
    c[8] = -CLIP
    return c


_CACHE = {}


def _get_module(n_cores=NCORES):
    if n_cores not in _CACHE:
        _CACHE[n_cores] = build_module(n_cores)
    return _CACHE[n_cores]


def make_in_maps(inputs, n_cores=NCORES):
    rows = N // n_cores
    y_true = np.ascontiguousarray(np.asarray(inputs["y_true"], np.float32).reshape(N, 1))
    y_pred = np.ascontiguousarray(np.asarray(inputs["y_pred"], np.float32).reshape(N, 1))
    zi0 = np.ascontiguousarray(np.asarray(inputs["Z_idx0"]).astype(np.int32).reshape(N))
    zi1 = np.ascontiguousarray(np.asarray(inputs["Z_idx1"]).astype(np.int32).reshape(N))
    c = host_consts(np.asarray(inputs["sig2e"]), np.asarray(inputs["sig2bs"], np.float64))
    cs = cheb_coeffs().astype(np.float32)
    import ml_dtypes
    hi_ = cs.astype(ml_dtypes.bfloat16).astype(np.float32)
    lo_ = (cs - hi_).astype(ml_dtypes.bfloat16).astype(np.float32)
    chebc = np.concatenate([hi_, lo_])
    nch = rows // 128
    maps = []
    for i in range(n_cores):
        sl = slice(i * rows, (i + 1) * rows)
        pk = np.concatenate([
            y_true[sl].reshape(nch, 128).T,
            y_pred[sl].reshape(nch, 128).T,
            zi0[sl].reshape(nch, 128).T.view(np.float32),
            zi1[sl].reshape(nch, 128).T.view(np.float32),
        ], axis=1)
        maps.append({
            "packed": np.ascontiguousarray(pk),
            "consts": c, "chebc": chebc,
        })
    return maps


def kernel(**inputs):
    nc = _get_module(NCORES)
    maps = make_in_maps(inputs, NCORES)
    res = run_bass_kernel_spmd(nc, maps, list(range(NCORES)))
    out = np.asarray(res.results[0]["out"], np.float32).reshape(1, 1)
    return out


# revision 25
# speedup vs baseline: 1.0780x; 1.0780x over previous
"""COPNLL loss kernel for Trainium2 (8 NeuronCores).

Math: the reference builds V = (sig2e*I + sig2bs0*Z0 Z0^T + sig2bs1*Z1 Z1^T)/sig2
with Z0 (4096x1000), Z1 (4096x500) one-hot, then needs logdet(V) and m^T V^-1 m.
Both reduce via Woodbury to the 1500x1500 capacitance matrix
    K = [[sig2e/s0*I + diag(c0), C], [C^T, sig2e/s1*I + diag(c1)]]
whose (0,0) block is diagonal, leaving one dense 500x500 Schur complement
    S = (sig2e/s1*I + diag(c1)) - C^T diag(1/A) C,   A = sig2e/s0 + c0
with C = Z0^T Z1 (co-occurrence counts), c0/c1 level counts, a = Z0^T m, b = Z1^T m:
    logdet(sig2*V) = (N-q)log sig2e + q0 log s0 + q1 log s1 + sum(log A) + logdet S
    m^T V^-1 m     = (sig2/sig2e) * (m^T m - a^T A^-1 a - t^T S^-1 t),
                     t = b - C^T (a/A)
Device plan (SPMD on 8 cores):
  phase A (row-sharded, 512 rows/core): one-hot chunks via iota+is_equal;
    an early pass computes G1 = [1|m]^T [Z1|1|m] and G0a = [1|m]^T Z0, whose
    f32 all-reduce overlaps the main C = Z0^T Z1 matmuls; C is all-reduced
    in int8 (entries are small co-occurrence counts - exact).
  phase C (redundant on all cores): assemble S (padded to 512), block LDL with
    128-blocks; block inverses via Hotelling/Newton-Schulz Y-chains (spectrum
    of S ~ [1.7,17]) applied directly to the trailing panel; block logdets via
    Chebyshev trace of log with the weighted sum accumulated on the PE.
"""

import math
import sys
import types

import numpy as np

import concourse.bass as bass
import concourse.bacc as bacc
import concourse.mybir as mybir
from concourse.bass import ds, ts
from concourse.bass_utils import run_bass_kernel_spmd
from concourse.masks import make_identity
from concourse.tile import TileContext


def _ensure_axon_hooks():
    """bass_utils imports antenv.axon_hooks when tracing; this image's antenv
    lacks it. Provide a shim (with the real ctypes NTFF hook when available)
    so trace=True/BASS_TRACE never crashes the kernel."""
    try:
        import antenv.axon_hooks  # noqa: F401
        return
    except ImportError:
        pass
    try:
        import trn_agent_boot.trn_boot as tb
        hook = tb._ntff_profile_via_ctypes("/opt/axon/libaxon_pjrt.so")
    except Exception:
        hook = None
    mod = types.ModuleType("antenv.axon_hooks")
    mod._hook = hook
    mod.get_axon_ntff_profile_hook = lambda: mod._hook

    def _set(h):
        mod._hook = h

    mod.set_axon_ntff_profile_hook = _set
    sys.modules["antenv.axon_hooks"] = mod
    try:
        import antenv
        antenv.axon_hooks = mod
    except ImportError:
        pass
    try:
        import concourse.bass_utils as bu
        _orig_upload = bu.upload_artifacts

        def _safe_upload(tmpdir):
            try:
                return _orig_upload(tmpdir)
            except Exception:
                return f"local:{tmpdir}"

        bu.upload_artifacts = _safe_upload
    except Exception:
        pass


_ensure_axon_hooks()

N = 4096
NCORES = 8
Q0, Q0P = 1000, 1024
T0 = Q0P // 128            # 8 level-0 tiles of 128
Q1 = 500
FR = Q1 + 2                # rhs width: [Z1 | 1 | m]
SP = 512                   # padded S size
NBLK = SP // 128           # 4
W3 = Q1 - 3 * 128          # 116: valid width of the last S block
LO, HI = 1.4, 18.0         # eigenvalue bounds for NS init + Chebyshev interval
NS_ITERS = 5
CHEB_DEG = 12
NCOEF = CHEB_DEG + 1
CLIP = 4.2648907939226017  # sqrt(2)*erfinv(1-2e-5)

F32 = mybir.dt.float32
BF16 = mybir.dt.bfloat16
I8 = mybir.dt.int8
I32 = mybir.dt.int32
AX = mybir.AxisListType
OP = mybir.AluOpType
ACT = mybir.ActivationFunctionType

RED_C = 128 * T0 * Q1           # int8 elements: the C matrix
REDF_G0A = 2 * Q0P              # f32: [counts0 ; a] rows
REDF_G1 = REDF_G0A + 2 * FR     # f32: + G1
REDF_N = REDF_G1 + 2            # f32: + scalars


def cheb_coeffs(lo=LO, hi=HI, deg=CHEB_DEG):
    K = 4000
    th = (np.arange(K) + 0.5) * np.pi / K
    xk = np.cos(th)
    fk = np.log((hi - lo) / 2.0 * xk + (hi + lo) / 2.0)
    cs = np.array([2.0 / K * np.sum(fk * np.cos(j * th)) for j in range(deg + 1)])
    cs[0] *= 0.5
    return cs.astype(np.float32)


def _diag_fill(nc, tile_ap, value):
    nc.gpsimd.memset(tile_ap, 0.0)
    nc.gpsimd.affine_select(out=tile_ap, in_=tile_ap, compare_op=OP.not_equal,
                            fill=value, base=0, pattern=[[-1, 128]],
                            channel_multiplier=1)


def build_module(n_cores=NCORES):
    rows = N // n_cores
    nch = rows // 128          # 128-row chunks per core

    nc = bacc.Bacc(num_devices=n_cores)
    pk_d = nc.declare_dram_parameter("packed", [128, 4 * (rows // 128)], F32,
                                     isOutput=False)
    cst_d = nc.declare_dram_parameter("consts", [16], F32, isOutput=False)
    chb_d = nc.declare_dram_parameter("chebc", [2 * NCOEF], F32, isOutput=False)
    out_d = nc.declare_dram_parameter("out", [1, 1], F32, isOutput=True)

    redb_in = nc.dram_tensor("redb_in", [RED_C], I8)
    redb_out = nc.dram_tensor("redb_out", [RED_C], I8, addr_space="Shared")
    redf_in = nc.dram_tensor("redf_in", [REDF_N], F32)
    redf_out = nc.dram_tensor("redf_out", [REDF_N], F32, addr_space="Shared")
    warm_in = nc.dram_tensor("warm_in", [16], F32)
    warm_out = nc.dram_tensor("warm_out", [16], F32, addr_space="Shared")

    with TileContext(nc) as tc, \
         tc.tile_pool(name="consts", bufs=1) as consts, \
         tc.tile_pool(name="work", bufs=1) as work:

        # ---- constants ----
        ident = consts.tile([128, 128], F32, tag="ident")
        make_identity(nc, ident)
        i2 = consts.tile([128, 128], F32, tag="i2")              # 2*I
        _diag_fill(nc, i2, 2.0)
        alphaI = consts.tile([128, 128], F32, tag="alphaI")      # NS init
        _diag_fill(nc, alphaI, 2.0 / (LO + HI))
        shiftI = consts.tile([128, 128], F32, tag="shiftI")      # Chebyshev shift
        _diag_fill(nc, shiftI, (HI + LO) / (HI - LO))
        ones512 = consts.tile([128, SP], F32, tag="ones512")
        nc.vector.memset(ones512, 1.0)
        identB16 = consts.tile([128, 128], BF16, tag="identB16")
        nc.vector.tensor_copy(identB16, ident)

        cst_row = consts.tile([1, 16], F32, tag="cst_row")
        nc.sync.dma_start(cst_row, cst_d[:].rearrange("(p x) -> p x", p=1))
        cst_row2 = consts.tile([1, 16], F32, tag="cst_row2")
        nc.vector.tensor_copy(cst_row2, cst_row)
        chb = consts.tile([1, 2 * NCOEF], F32, tag="chb")
        nc.sync.dma_start(chb, chb_d[:].rearrange("(p x) -> p x", p=1))
        chb2 = consts.tile([1, 2 * NCOEF], F32, tag="chb2")
        nc.vector.tensor_copy(chb2, chb)
        cst = consts.tile([128, 16], F32, tag="cst")
        chbB = consts.tile([128, 2 * NCOEF], F32, tag="chbB")
        with tc.tile_pool(name="setup_ps", bufs=2,
                          space=bass.MemorySpace.PSUM) as gps0:
            ps_b = gps0.tile([128, 16], F32, tag="gps0")
            nc.tensor.matmul(ps_b, ones512[0:1, 0:128], cst_row2,
                             start=True, stop=True)
            nc.vector.tensor_copy(cst, ps_b)
            ps_c = gps0.tile([128, 2 * NCOEF], F32, tag="gps0")
            nc.tensor.matmul(ps_c, ones512[0:1, 0:128], chb2,
                             start=True, stop=True)
            nc.vector.tensor_copy(chbB, ps_c)

        # iotas for the one-hot compares
        iota0i = work.tile([128, Q0P], I32, tag="iota0i")
        nc.gpsimd.iota(iota0i, pattern=[[1, Q0P]], base=0, channel_multiplier=0)
        iota0 = work.tile([128, Q0P], F32, tag="iota0")
        nc.vector.tensor_copy(iota0, iota0i)
        iota1i = work.tile([128, Q1], I32, tag="iota1i")
        nc.gpsimd.iota(iota1i, pattern=[[1, Q1]], base=0, channel_multiplier=0)
        iota1 = work.tile([128, Q1], F32, tag="iota1")
        nc.vector.tensor_copy(iota1, iota1i)

        # ---- inputs -> m, sum r^2, sum m^2 ----
        packed = work.tile([128, 4 * nch], F32, tag="packed")
        nc.sync.dma_start(packed, pk_d[:])
        yt = packed[:, 0:nch]
        yp = packed[:, nch:2 * nch]
        idx0 = work.tile([128, nch], F32, tag="idx0")
        nc.vector.tensor_copy(idx0, packed[:, 2 * nch:3 * nch].bitcast(I32))
        idx1 = work.tile([128, nch], F32, tag="idx1")
        nc.vector.tensor_copy(idx1, packed[:, 3 * nch:4 * nch].bitcast(I32))
        resid = work.tile([128, nch], F32, tag="resid")
        nc.vector.tensor_sub(resid, yt, yp)
        mvec = work.tile([128, nch], F32, tag="mvec")
        nc.vector.tensor_scalar(out=mvec, in0=resid, scalar1=cst[:, 0:1],
                                scalar2=cst[:, 1:2], op0=OP.mult, op1=OP.min)
        nc.vector.tensor_scalar(out=mvec, in0=mvec, scalar1=cst[:, 8:9],
                                scalar2=None, op0=OP.max)
        scr_n = work.tile([128, nch], F32, tag="scr_n")
        scal2 = work.tile([128, 2], F32, tag="scal2")
        nc.vector.tensor_mul(scr_n, resid, resid)
        nc.vector.tensor_reduce(scal2[:, 0:1], scr_n, AX.X, OP.add)
        nc.vector.tensor_mul(scr_n, mvec, mvec)
        nc.vector.tensor_reduce(scal2[:, 1:2], scr_n, AX.X, OP.add)
        scal2r = work.tile([1, 2], F32, tag="scal2r")
        with tc.tile_pool(name="sc_ps", bufs=1,
                          space=bass.MemorySpace.PSUM) as gpsc:
            ps_s = gpsc.tile([128, 2], F32, tag="gpsc")
            nc.tensor.matmul(ps_s[0:1, 0:2], ones512[:, 0:1], scal2,
                             start=True, stop=True)
            nc.vector.tensor_copy(scal2r, ps_s[0:1, 0:2])

        # ---- phase A pass 1: G1 = [1|m]^T [Z1|1|m], G0a = [1|m]^T Z0 ----
        G1 = work.tile([2, FR], F32, tag="G1")
        G0a = work.tile([2, Q0P], F32, tag="G0a")
        share = nch <= 4
        oh0_keep, rhs_keep = [], []
        with (
            tc.tile_pool(name="ph1", bufs=(1 if share else 2)) as ph1,
            tc.tile_pool(name="ph1_ps", bufs=1, space=bass.MemorySpace.PSUM) as pg1,
        ):
            psg1 = pg1.tile([2, FR], F32, tag="psg1")
            psa0 = pg1.tile([2, SP], F32, tag="psa0")
            psa1 = pg1.tile([2, SP], F32, tag="psa1")
            for c in range(nch):
                cc = c % 4
                pool_c = work if share else ph1
                oh = pool_c.tile([128, Q0P], BF16, tag=f"oh0_{cc if not share else c}",
                                 name=f"oh0w_{c if share else cc}")
                nc.vector.tensor_scalar(out=oh, in0=iota0,
                                        scalar1=idx0[:, c:c + 1],
                                        scalar2=None, op0=OP.is_equal)
                rh = pool_c.tile([128, FR], BF16, tag=f"rhs_{cc if not share else c}",
                                 name=f"rhsw_{c if share else cc}")
                nc.vector.tensor_scalar(out=rh[:, 0:Q1], in0=iota1,
                                        scalar1=idx1[:, c:c + 1],
                                        scalar2=None, op0=OP.is_equal)
                nc.vector.memset(rh[:, Q1:Q1 + 1], 1.0)
                nc.vector.tensor_copy(rh[:, Q1 + 1:FR], mvec[:, c:c + 1])
                if share:
                    oh0_keep.append(oh)
                    rhs_keep.append(rh)
                first, lastc = c == 0, c == nch - 1
                nc.tensor.matmul(psg1, rh[:, Q1:FR], rh, start=first, stop=lastc)
                nc.tensor.matmul(psa0, rh[:, Q1:FR], oh[:, 0:SP],
                                 start=first, stop=lastc)
                nc.tensor.matmul(psa1, rh[:, Q1:FR], oh[:, SP:Q0P],
                                 start=first, stop=lastc)
            nc.vector.tensor_copy(G1, psg1)
            nc.vector.tensor_copy(G0a[:, 0:SP], psa0)
            nc.vector.tensor_copy(G0a[:, SP:Q0P], psa1)

        # early f32 all-reduce: G0a + G1 + scalars (overlaps the C matmuls)
        nc.sync.dma_start(
            redf_in[0:REDF_G0A].rearrange("(p f) -> p f", p=2), G0a)
        nc.sync.dma_start(
            redf_in[REDF_G0A:REDF_G1].rearrange("(p f) -> p f", p=2), G1)
        nc.sync.dma_start(
            redf_in[REDF_G1:REDF_G1 + 2].rearrange("(p f) -> p f", p=1), scal2r)
        if n_cores > 1:
            nc.gpsimd.collective_compute(
                "AllReduce", OP.add,
                replica_groups=[list(range(n_cores))],
                ins=[redf_in[:]], outs=[redf_out[:]],
            )
        else:
            nc.sync.dma_start(redf_out[:], redf_in[:])

        # phase C constants built early (fills the collective wait)
        iotaLi = work.tile([128, T0], I32, tag="iotaLi")
        nc.gpsimd.iota(iotaLi, pattern=[[128, T0]], base=0, channel_multiplier=1)
        iotaL = work.tile([128, T0], F32, tag="iotaL")
        nc.vector.tensor_copy(iotaL, iotaLi)
        padmask = work.tile([128, T0], mybir.dt.uint32, tag="padmask")
        nc.vector.tensor_scalar(out=padmask, in0=iotaL, scalar1=float(Q0) - 0.5,
                                scalar2=None, op0=OP.is_gt)
        cIh, cIl = [], []
        for j in range(NCOEF):
            th_ = work.tile([128, 128], BF16, tag=f"cIh{j}", name=f"cIh{j}")
            nc.vector.tensor_scalar_mul(th_, ident, chbB[:, j:j + 1])
            cIh.append(th_)
            tl_ = work.tile([128, 128], BF16, tag=f"cIl{j}", name=f"cIl{j}")
            nc.vector.tensor_scalar_mul(
                tl_, ident, chbB[:, NCOEF + j:NCOEF + j + 1])
            cIl.append(tl_)

        # ---- phase A pass 2: C = Z0^T Z1 ----
        G0 = work.tile([128, T0, Q1], BF16, tag="G0")
        with (
            tc.tile_pool(name="phA", bufs=2) as pha,
            tc.tile_pool(name="phA_ps", bufs=1, space=bass.MemorySpace.PSUM) as pps,
        ):
            psa = [pps.tile([128, Q1], F32, tag=f"psa{t}", name=f"psa{t}")
                   for t in range(T0)]
            for c in range(nch):
                cc = c % 4
                if share:
                    oh, rh = oh0_keep[c], rhs_keep[c]
                else:
                    oh = pha.tile([128, Q0P], BF16, tag=f"oh0b_{cc}",
                                  name=f"oh0b_{cc}")
                    nc.vector.tensor_scalar(out=oh, in0=iota0,
                                            scalar1=idx0[:, c:c + 1],
                                            scalar2=None, op0=OP.is_equal)
                    rh = pha.tile([128, FR], BF16, tag=f"rhsb_{cc}",
                                  name=f"rhsb_{cc}")
                    nc.vector.tensor_scalar(out=rh[:, 0:Q1], in0=iota1,
                                            scalar1=idx1[:, c:c + 1],
                                            scalar2=None, op0=OP.is_equal)
                for t in range(T0):
                    nc.tensor.matmul(psa[t], oh[:, ts(t, 128)], rh[:, 0:Q1],
                                     start=(c == 0), stop=(c == nch - 1))
            for t in range(T0):
                nc.vector.tensor_copy(G0[:, t, :], psa[t])

        # ---- int8 all-reduce of C (co-occurrence counts: exact) ----
        C8 = work.tile([128, T0, Q1], I8, tag="C8")
        nc.vector.tensor_copy(C8, G0)
        nc.sync.dma_start(
            redb_in[:].rearrange("(p t f) -> p t f", p=128, t=T0), C8)
        if n_cores > 1:
            nc.gpsimd.collective_compute(
                "AllReduce", OP.add,
                replica_groups=[list(range(n_cores))],
                ins=[redb_in[:]], outs=[redb_out[:]],
            )
        else:
            nc.sync.dma_start(redb_out[:], redb_in[:])
        # f32 results land first; their dependent work overlaps the int8 AR
        nc.sync.dma_start(
            G0a, redf_out[0:REDF_G0A].rearrange("(p f) -> p f", p=2))
        nc.sync.dma_start(
            G1, redf_out[REDF_G0A:REDF_G1].rearrange("(p f) -> p f", p=2))
        r2g = work.tile([1, 1], F32, tag="r2g")
        nc.sync.dma_start(
            r2g, redf_out[REDF_G1:REDF_G1 + 1].rearrange("(p f) -> p f", p=1))
        mtm = work.tile([1, 1], F32, tag="mtm")
        nc.sync.dma_start(
            mtm, redf_out[REDF_G1 + 1:REDF_G1 + 2].rearrange("(p f) -> p f", p=1))
        nc.sync.dma_start(
            C8, redb_out[:].rearrange("(p t f) -> p t f", p=128, t=T0))
        nc.vector.tensor_copy(G0, C8)

        # ---- phase C: S assembly ----
        Srow = [work.tile([128, SP], BF16, tag=f"Srow{i}", name=f"Srow{i}")
                for i in range(NBLK)]
        zvec = [work.tile([128, 1], F32, tag=f"z{i}", name=f"z{i}")
                for i in range(NBLK)]
        Avec = work.tile([128, T0], F32, tag="Avec")
        aAll = work.tile([128, T0], F32, tag="aAll")
        Winv = work.tile([128, T0], F32, tag="Winv")
        Cw = work.tile([128, T0, Q1], BF16, tag="Cw")

        with tc.tile_pool(name="sasm_ps", bufs=2,
                          space=bass.MemorySpace.PSUM) as sps:
            # counts0/a -> per-partition layout via PE transposes
            for t in range(T0):
                psT = sps.tile([128, 2], F32, tag="pst")
                nc.tensor.transpose(psT, G0a[0:2, ts(t, 128)], ident[0:2, 0:2])
                nc.vector.tensor_copy(Avec[:, t:t + 1], psT[:, 0:1])
                nc.vector.tensor_copy(aAll[:, t:t + 1], psT[:, 1:2])
            nc.vector.tensor_scalar(out=Avec, in0=Avec, scalar1=cst[:, 2:3],
                                    scalar2=None, op0=OP.add)
            nc.vector.copy_predicated(Avec, padmask, ones512[:, 0:T0])
            nc.vector.reciprocal(Winv, Avec)
            scr_t = work.tile([128, T0], F32, tag="scr_t")
            logA = work.tile([128, 1], F32, tag="logA")
            nc.scalar.activation(scr_t, Avec, ACT.Ln, accum_out=logA)
            for t in range(T0):
                nc.vector.tensor_scalar_mul(Cw[:, t, :], G0[:, t, :],
                                             Winv[:, t:t + 1])
            # quad_a = sum(a^2 / A)
            qa = work.tile([128, 1], F32, tag="qa")
            qscr = work.tile([128, T0], F32, tag="qscr")
            nc.vector.tensor_mul(qscr, aAll, aAll)
            nc.vector.tensor_mul(qscr, qscr, Winv)
            nc.vector.tensor_reduce(qa, qscr, AX.X, OP.add)
            aW = work.tile([128, T0], BF16, tag="aW")
            nc.vector.tensor_mul(aW, aAll, Winv)

            for i in range(NBLK):
                wi = 128 if i < NBLK - 1 else W3
                pss = sps.tile([128, Q1], F32, tag="pss", bufs=4)
                for t in range(T0):
                    nc.tensor.matmul(pss[:wi, :], Cw[:, t, ds(i * 128, wi)],
                                     G0[:, t, :], start=(t == 0),
                                     stop=(t == T0 - 1))
                # (C^T a/A)_i for the t vector
                psta = sps.tile([128, 1], F32, tag="psta", bufs=2)
                for t in range(T0):
                    nc.tensor.matmul(psta[:wi, :], G0[:, t, ds(i * 128, wi)],
                                     aW[:, t:t + 1],
                                     start=(t == 0), stop=(t == T0 - 1))
                nc.vector.memset(Srow[i], 0.0)
                nc.vector.tensor_scalar_mul(Srow[i][:wi, 0:Q1], pss[:wi, 0:Q1],
                                            -1.0)
                # c1/b block via PE transpose of G1[0:2, block]
                psT = sps.tile([128, 2], F32, tag="pst")
                nc.tensor.transpose(psT[:wi, :], G1[0:2, ds(i * 128, wi)],
                                    ident[0:2, 0:2])
                cbt = work.tile([128, 2], F32, tag=f"cb{i}", name=f"cb{i}")
                nc.vector.memset(cbt, 0.0)
                nc.vector.tensor_copy(cbt[:wi, :], psT[:wi, :])
                dS = work.tile([128, 1], F32, tag=f"dS{i}", name=f"dS{i}")
                nc.vector.tensor_scalar(out=dS, in0=cbt[:, 0:1],
                                        scalar1=cst[:, 3:4],
                                        scalar2=None, op0=OP.add)
                if i == NBLK - 1:
                    pm3 = work.tile([128, 1], mybir.dt.uint32, tag="pm3")
                    nc.vector.tensor_scalar(out=pm3, in0=iotaL[:, 0:1],
                                            scalar1=float(W3) - 0.5,
                                            scalar2=None, op0=OP.is_gt)
                    nc.vector.copy_predicated(dS, pm3, ones512[:, 0:1])
                dgblk = work.tile([128, 128], BF16, tag="dgblk")
                nc.vector.tensor_scalar_mul(dgblk, ident, dS)
                nc.vector.tensor_add(Srow[i][:, ts(i, 128)],
                                     Srow[i][:, ts(i, 128)], dgblk)

                nc.vector.memset(zvec[i], 0.0)
                nc.vector.tensor_sub(zvec[i][:wi, :], cbt[:wi, 1:2],
                                     psta[:wi, 0:1])

        # ---- block LDL: Hotelling chains + deferred Chebyshev traces ----
        Binv = [work.tile([128, 128], F32, tag=f"Binv{k}", name=f"Binv{k}")
                for k in range(NBLK)]
        Wk = [work.tile([128, SP - (k + 1) * 128], BF16, tag=f"Wk{k}",
                        name=f"Wk{k}") for k in range(NBLK - 1)]
        Wk32 = [work.tile([128, SP - (k + 1) * 128], F32, tag=f"Wk32_{k}",
                          name=f"Wk32_{k}") for k in range(NBLK - 1)]
        trc = work.tile([128, NBLK], F32, tag="trc")
        qtt = work.tile([128, NBLK], F32, tag="qtt")

        with (
            tc.tile_pool(name="ldl", bufs=4) as ldl,
            tc.tile_pool(name="ldl_ps", bufs=4, space=bass.MemorySpace.PSUM) as lps,
        ):
            alpha = 2.0 / (LO + HI)
            for k in range(NBLK):
                Bk = Srow[k][:, ts(k, 128)]
                # Hotelling: Y' = Y Z, Z = 2I - Y, Y0 = alpha*B -> Y -> I
                # X trails (-> B^-1); V trails on the panel (-> B^-1 Strail)
                trail = SP - (k + 1) * 128 if k < NBLK - 1 else 0
                Y = ldl.tile([128, 128], BF16, tag="nsY")
                nc.vector.tensor_scalar_mul(Y, Bk, alpha)
                Z = ldl.tile([128, 128], BF16, tag="nsZ")
                nc.vector.tensor_sub(Z, i2, Y)
                X = ldl.tile([128, 128], BF16, tag="nsX")
                nc.vector.tensor_copy(X, alphaI)
                psX = None
                for it in range(NS_ITERS):
                    last = it == NS_ITERS - 1
                    if not last:
                        psY = lps.tile([128, 128], F32, tag="lps")
                        nc.tensor.matmul(psY, Y, Z, start=True, stop=True)
                    psX = lps.tile([128, 128], F32, tag="lps")
                    nc.tensor.matmul(psX, X, Z, start=True, stop=True)
                    X = ldl.tile([128, 128], BF16, tag="nsX")
                    nc.vector.tensor_copy(X, psX)
                    if not last:
                        Z = ldl.tile([128, 128], BF16, tag="nsZ")
                        nc.vector.tensor_sub(Z, i2, psY)
                        Y = ldl.tile([128, 128], BF16, tag="nsY")
                        nc.vector.tensor_copy(Y, psY)
                nc.vector.tensor_copy(Binv[k], psX)
                if trail:
                    psW = lps.tile([128, 384], F32, tag="lps")
                    nc.tensor.matmul(psW[:, :trail], X,
                                     Srow[k][:, (k + 1) * 128:SP],
                                     start=True, stop=True)
                    nc.vector.tensor_copy(Wk[k], psW[:, :trail])
                    nc.vector.tensor_copy(Wk32[k], psW[:, :trail])
                    for i in range(k + 1, NBLK):
                        psu = lps.tile([128, 384], F32, tag="lps")
                        nc.tensor.matmul(psu[:, :trail], Srow[k][:, ts(i, 128)],
                                         Wk[k], start=True, stop=True)
                        nc.vector.tensor_sub(Srow[i][:, (k + 1) * 128:SP],
                                             Srow[i][:, (k + 1) * 128:SP],
                                             psu[:, :trail])

            # forward substitution: z_i -= (Wk[k] block i)^T z_k
            for k in range(NBLK - 1):
                for i in range(k + 1, NBLK):
                    psz = lps.tile([128, 1], F32, tag="lps")
                    off = (i - k - 1) * 128
                    nc.tensor.matmul(psz, Wk32[k][:, ds(off, 128)], zvec[k],
                                     start=True, stop=True)
                    nc.vector.tensor_sub(zvec[i], zvec[i], psz)
            # quad_t = sum_k z_k^T Binv_k z_k
            for k in range(NBLK):
                psq = lps.tile([128, 1], F32, tag="lps")
                nc.tensor.matmul(psq, Binv[k], zvec[k], start=True, stop=True)
                uk = ldl.tile([128, 1], F32, tag="uk")
                nc.vector.tensor_copy(uk, psq)
                nc.vector.tensor_mul(qtt[:, k:k + 1], zvec[k], uk)

            # Chebyshev trace chains, 4-wide interleaved; the weighted sum
            # R_k = sum_j c_j T_j accumulates on the PE via stationary c_j*I
            b2s, tprevs, tcurs, Rps = [], [], [], []
            for k in range(NBLK):
                Bk = Srow[k][:, ts(k, 128)]
                bh = ldl.tile([128, 128], BF16, tag=f"bh{k}", name=f"bh{k}")
                nc.vector.tensor_scalar_mul(bh, Bk, 2.0 / (HI - LO))
                nc.vector.tensor_sub(bh, bh, shiftI)
                b2 = ldl.tile([128, 128], BF16, tag=f"b2{k}", name=f"b2{k}")
                nc.vector.tensor_scalar_mul(b2, bh, 2.0)
                b2s.append(b2)
                tprev = ldl.tile([128, 128], BF16, tag=f"chT{k}",
                                 name=f"chTp{k}", bufs=3)
                nc.vector.tensor_copy(tprev, identB16)
                tprevs.append(tprev)
                tcurs.append(bh)
                R = lps.tile([128, 128], F32, tag="Rps", bufs=4,
                             name=f"Rps{k}")
                Rps.append(R)
                nc.tensor.matmul(R, cIh[0], identB16, start=True, stop=False)
                nc.tensor.matmul(R, cIl[0], identB16, start=False, stop=False)
                nc.tensor.matmul(R, cIh[1], bh, start=False, stop=False)
                nc.tensor.matmul(R, cIl[1], bh, start=False, stop=False)
            for j in range(2, CHEB_DEG + 1):
                for k in range(NBLK):
                    psc = lps.tile([128, 128], F32, tag="lps")
                    nc.tensor.matmul(psc, b2s[k], tcurs[k], start=True,
                                     stop=True)
                    tnext = ldl.tile([128, 128], BF16, tag=f"chT{k}",
                                     name=f"chT{k}_{j}", bufs=3)
                    nc.vector.tensor_sub(tnext, psc, tprevs[k])
                    nc.tensor.matmul(Rps[k], cIh[j], tnext, start=False,
                                     stop=False)
                    nc.tensor.matmul(Rps[k], cIl[j], tnext, start=False,
                                     stop=(j == CHEB_DEG))
                    tprevs[k], tcurs[k] = tcurs[k], tnext
            for k in range(NBLK):
                Rsb = ldl.tile([128, 128], F32, tag="Rsb")
                nc.vector.tensor_mul(Rsb, Rps[k], ident)   # keep diagonal only
                nc.vector.tensor_reduce(trc[:, k:k + 1], Rsb, AX.X, OP.add)

        # ---- final scalar assembly ----
        qtr = work.tile([128, 1], F32, tag="qtr")
        nc.vector.tensor_reduce(qtr, qtt, AX.X, OP.add)
        smalls_c = work.tile([128, 3 + NBLK], F32, tag="smalls_c")
        nc.vector.tensor_copy(smalls_c[:, 0:1], logA)
        nc.vector.tensor_copy(smalls_c[:, 1:2], qa)
        nc.vector.tensor_copy(smalls_c[:, 2:3], qtr)
        nc.vector.tensor_copy(smalls_c[:, 3:3 + NBLK], trc)
        smalls = work.tile([1, 3 + NBLK], F32, tag="smalls")
        ldS = work.tile([1, 1], F32, tag="ldS")
        with tc.tile_pool(name="fin_ps", bufs=1,
                          space=bass.MemorySpace.PSUM) as gps2:
            ps_sm = gps2.tile([128, 3 + NBLK], F32, tag="gps2")
            nc.tensor.matmul(ps_sm[0:1, :], ones512[:, 0:1], smalls_c,
                             start=True, stop=True)
            nc.vector.tensor_copy(smalls, ps_sm[0:1, :])
        nc.vector.tensor_reduce(ldS, smalls[:, 3:3 + NBLK], AX.X, OP.add)

        fin = work.tile([1, 8], F32, tag="fin")
        # quadK = quad_a + quad_t
        nc.vector.tensor_add(fin[:, 0:1], smalls[:, 1:2], smalls[:, 2:3])
        # mVinvm = (sig2/sig2e) * (mtm - quadK)
        nc.vector.tensor_sub(fin[:, 1:2], mtm, fin[:, 0:1])
        nc.vector.tensor_scalar_mul(fin[:, 1:2], fin[:, 1:2], cst[0:1, 6:7])
        # logdetV = const1 + sum log A + logdet S
        nc.vector.tensor_add(fin[:, 2:3], smalls[:, 0:1], ldS)
        nc.vector.tensor_scalar(out=fin[:, 2:3], in0=fin[:, 2:3],
                                scalar1=cst[0:1, 4:5], scalar2=None, op0=OP.add)
        # sum_log_pdf = const2 - sum_r2/(2 sig2)
        nc.vector.tensor_scalar(out=fin[:, 3:4], in0=r2g, scalar1=cst[0:1, 7:8],
                                scalar2=cst[0:1, 5:6], op0=OP.mult, op1=OP.add)
        # total = 0.5*(logdetV + mVinvm - mtm + sum_log_pdf)
        nc.vector.tensor_add(fin[:, 4:5], fin[:, 2:3], fin[:, 1:2])
        nc.vector.tensor_sub(fin[:, 4:5], fin[:, 4:5], mtm)
        nc.vector.tensor_add(fin[:, 4:5], fin[:, 4:5], fin[:, 3:4])
        nc.vector.tensor_scalar_mul(fin[:, 4:5], fin[:, 4:5], 0.5)

        nc.sync.dma_start(out_d[:], fin[:, 4:5])

    nc.finalize()
    return nc


def host_consts(sig2e, sig2bs):
    s0, s1 = float(sig2bs[0]), float(sig2bs[1])
    sig2e = float(sig2e)
    sig2 = sig2e + s0 + s1
    c = np.zeros(16, np.float32)
    c[0] = 1.0 / math.sqrt(sig2)
    c[1] = CLIP
    c[2] = sig2e / s0
    c[3] = sig2e / s1
    c[4] = ((N - Q0 - Q1) * math.log(sig2e) + Q0 * math.log(s0)
            + Q1 * math.log(s1) - N * math.log(sig2))
    c[5] = -0.5 * N * math.log(2.0 * math.pi * sig2)
    c[6] = sig2 / sig2e
    c[7] = -1.0 / (2.0 * sig2)
    c[8] = -CLIP
    return c


_CACHE = {}


def _get_module(n_cores=NCORES):
    if n_cores not in _CACHE:
        _CACHE[n_cores] = build_module(n_cores)
    return _CACHE[n_cores]


def make_in_maps(inputs, n_cores=NCORES):
    rows = N // n_cores
    y_true = np.ascontiguousarray(np.asarray(inputs["y_true"], np.float32).reshape(N, 1))
    y_pred = np.ascontiguousarray(np.asarray(inputs["y_pred"], np.float32).reshape(N, 1))
    zi0 = np.ascontiguousarray(np.asarray(inputs["Z_idx0"]).astype(np.int32).reshape(N))
    zi1 = np.ascontiguousarray(np.asarray(inputs["Z_idx1"]).astype(np.int32).reshape(N))
    c = host_consts(np.asarray(inputs["sig2e"]), np.asarray(inputs["sig2bs"], np.float64))
    cs = cheb_coeffs().astype(np.float32)
    import ml_dtypes
    hi_ = cs.astype(ml_dtypes.bfloat16).astype(np.float32)
    lo_ = (cs - hi_).astype(ml_dtypes.bfloat16).astype(np.float32)
    chebc = np.concatenate([hi_, lo_])
    nch = rows // 128
    maps = []
    for i in range(n_cores):
        sl = slice(i * rows, (i + 1) * rows)
        pk = np.concatenate([
            y_true[sl].reshape(nch, 128).T,
            y_pred[sl].reshape(nch, 128).T,
            zi0[sl].reshape(nch, 128).T.view(np.float32),
            zi1[sl].reshape(nch, 128).T.view(np.float32),
        ], axis=1)
        maps.append({
            "packed": np.ascontiguousarray(pk),
            "consts": c, "chebc": chebc,
        })
    return maps


def kernel(**inputs):
    nc = _get_module(NCORES)
    maps = make_in_maps(inputs, NCORES)
    res = run_bass_kernel_spmd(nc, maps, list(range(NCORES)))
    out = np.asarray(res.results[0]["out"], np.float32).reshape(1, 1)
    return out
